# revision 2
# baseline (speedup 1.0000x reference)
"""Trainium2 Bass kernel for nn_MAD_72679436582977 (retrieval_knn).

For each edge endpoint (src/tgt of 1024 edges) and each of 4 heads: find the
8 nearest neighbors (excluding self) among 50000 nodes in a 32-d embedding
space, logits (q - e_k).f_q, dist |q - e_k|, softmax(1 - dist) over
16 neighbors + 8 sentinels, sigmoid of head-mean weighted sum.

Strategy: data-parallel over edges across 8 cores (128 edges/core, SPMD, no
collectives). Per core, per (head, endpoint) tile of 128 rows:
  - distance GEMM in float32r with K=34 (32 dims + |e|^2 row + |q|^2 row) so
    PSUM holds -d^2 directly; relevant values cluster near 0 where fp16 ULP
    is ~1e-3 (vs rank gaps ~5e-2) -> safe 16-bit candidate scans;
  - ACT (idle otherwise) converts PSUM fp32 -> SBUF fp16 (one full pass);
  - DVE block-max fold tree on fp16 at 2x rate: 2048 -> 256 block maxes of
    8 contiguous nodes (512+256+256 cycles vs 2x2168 for direct max8+index);
  - per 4096-node pair: max8 + find_index8 over 512 block maxes only;
  - top-9 elements provably live in the top-9 blocks by block max; take the
    global top-12 blocks of the 104 candidates (simulated worst-case output
    error 9e-3 incl. fp16 ties and f32r noise, vs 2e-2 budget);
  - indirect-DMA gather of the 12 winning blocks (8 contiguous rows of
    [embed|norm|pad] each) and exact fp32 recompute of s = 2 q.e - (qn+en)
    and u = e.f for all 96 candidates; exact top-9, drop rank-1 (self);
  - epilogue batched across all 8 tiles (single sqrt/exp ACT table loads):
    dist = sqrt(-s), weights exp(1-dist), softmax-ratio with sentinel mass,
    head mean, sigmoid.

Host only shards edges, lays out inputs, and concatenates the 8 per-core
outputs.
"""
import os
import sys

sys.path.insert(0, "/opt/trn_rl_repo")

import numpy as np

import concourse.bass as bass
import concourse.bacc as bacc
import concourse.mybir as mybir
from concourse import tile
from concourse.bass import IndirectOffsetOnAxis

F32 = mybir.dt.float32
F32R = mybir.dt.float32r
F16 = mybir.dt.float16
U32 = mybir.dt.uint32

N_HEADS = 4
N_NODES = 50000
DIM = 32
N_BATCH = 1024
N_SENT = 8
N_CORES = 8

EDGES_PER_CORE = N_BATCH // N_CORES          # 128
TILE_W = 2048                                 # PSUM tile (4 banks)
N_TILES = 25                                  # 24 full + 1 half
LAST_W = 1024
N_PAD = TILE_W * (N_TILES - 1) + LAST_W       # 50176
M_TILES = N_HEADS * 2                         # (head, src/tgt) x 128 rows
KC = DIM + 2                                  # 32 dims + en row + qn row
BLK = 8                                       # nodes per block
N_BLOCKS = N_PAD // BLK                       # 6272
N_PAIRS = 12                                  # pairs of 2048-tiles (4096 nodes)
PAIR_BLOCKS = 2 * (TILE_W // BLK)             # 512
N_CAND = N_PAIRS * 8 + 8                      # 104 candidate blocks
T_WIN = 12                                    # winning blocks kept
NCND = T_WIN * BLK                            # 96 candidate nodes
EW = DIM + 2                                  # embn row: embed|norm|pad
PAD_EN = 60000.0

LAST = {}


def _build_program():
    nc = bacc.Bacc(None, num_swdge_queues=2)

    rhs_d = nc.dram_tensor("rhs_aug", [N_HEADS, KC, N_PAD], F32R,
                           kind="ExternalInput")
    embn_d = nc.dram_tensor("embn", [N_HEADS * N_PAD, EW], F32,
                            kind="ExternalInput")
    qpack_d = nc.dram_tensor("qpack", [M_TILES, KC, 128], F32R,
                             kind="ExternalInput")
    aux_d = nc.dram_tensor("aux", [M_TILES, 128, 2 + 2 * DIM], F32,
                           kind="ExternalInput")

    preds_d = nc.dram_tensor("preds", [128, 1], F32, kind="ExternalOutput")
    dbg_gid_d = nc.dram_tensor("dbg_gid", [M_TILES, 128, NCND], U32,
                               kind="ExternalOutput")
    dbg_s_d = nc.dram_tensor("dbg_s", [M_TILES, 128, 8], F32,
                             kind="ExternalOutput")

    with tile.TileContext(nc) as tc:
        with tc.tile_pool(name="const", bufs=1) as cpool, \
             tc.tile_pool(name="qp", bufs=2) as qpool, \
             tc.tile_pool(name="rhs", bufs=3) as rpool, \
             tc.tile_pool(name="s16", bufs=3) as s16p, \
             tc.tile_pool(name="tree", bufs=2) as treep, \
             tc.tile_pool(name="bm", bufs=2) as bmpool, \
             tc.tile_pool(name="cand", bufs=2) as candp, \
             tc.tile_pool(name="gath", bufs=2) as gathp, \
             tc.tile_pool(name="prod", bufs=2) as prodp, \
             tc.tile_pool(name="small", bufs=3) as sp, \
             tc.tile_pool(name="acc", bufs=1) as accp, \
             tc.tile_pool(name="ps", bufs=2, space="PSUM") as psp:

            # constants
            iota_cand = cpool.tile([128, N_CAND], U32, tag="iota_cand")
            nc.gpsimd.iota(iota_cand[:], pattern=[[1, N_CAND]], base=0,
                           channel_multiplier=0)
            iota8f = cpool.tile([128, 8], F32, tag="iota8f")
            nc.gpsimd.iota(iota8f[:], pattern=[[1, 8]], base=0,
                           channel_multiplier=0,
                           allow_small_or_imprecise_dtypes=True)
            # per-pair block-gid offsets: gid = pair*512 + within (13th = solo)
            ioff = cpool.tile([128, N_PAIRS + 1, 8], F32, tag="ioff")
            nc.gpsimd.iota(ioff[:], pattern=[[PAIR_BLOCKS, N_PAIRS + 1], [0, 8]],
                           base=0, channel_multiplier=0,
                           allow_small_or_imprecise_dtypes=True)
            neg_inf8 = cpool.tile([128, 8], F32, tag="neg_inf8")
            nc.vector.memset(neg_inf8[:], -1e30)

            # retained per-m state for the batched epilogue
            s_all = accp.tile([128, M_TILES, NCND], F32, tag="s_all")
            u_all = accp.tile([128, M_TILES, NCND], F32, tag="u_all")
            mask_all = accp.tile([128, M_TILES, NCND], F32, tag="mask_all")
            qf_all = accp.tile([128, M_TILES], F32, tag="qf_all")

            for m in range(M_TILES):
                h = m // 2
                q_s = qpool.tile([KC, 128], F32R, tag="q")
                nc.sync.dma_start(out=q_s[:], in_=qpack_d[m])
                aux_s = qpool.tile([128, 2 + 2 * DIM], F32, tag="aux")
                nc.sync.dma_start(out=aux_s[:], in_=aux_d[m])
                qn_s = aux_s[:, 0:1]
                qf_s = aux_s[:, 1:2]
                qv_s = aux_s[:, 2:2 + DIM]
                fv_s = aux_s[:, 2 + DIM:2 + 2 * DIM]
                nc.vector.tensor_copy(qf_all[:, m:m + 1], qf_s)

                cand_v = candp.tile([128, N_CAND], F16, tag="cv")
                cand_i = candp.tile([128, N_CAND], U32, tag="ci")

                bm = None
                for j in range(N_TILES):
                    w = TILE_W if j < N_TILES - 1 else LAST_W
                    nb = w // BLK
                    rhs_s = rpool.tile([KC, w], F32R, tag="rhs")
                    nc.sync.dma_start(
                        out=rhs_s[:], in_=rhs_d[h, :, j * TILE_W:j * TILE_W + w])
                    psum = psp.tile([128, w], F32, tag="ps")
                    for b in range(w // 512):
                        sl = slice(b * 512, (b + 1) * 512)
                        nc.tensor.matmul(psum[:, sl], q_s[:], rhs_s[:, sl],
                                         start=True, stop=True)
                    # fp32 -> fp16 full pass on the (otherwise idle) scalar eng
                    s16 = s16p.tile([128, w], F16, tag="s16")
                    nc.scalar.activation(s16[:], psum[:],
                                         mybir.ActivationFunctionType.Copy,
                                         bias=0.0, scale=1.0)
                    # fold tree to per-block (8) maxes; f1/f2 run at 2x (fp16)
                    sv = s16[:].rearrange("p (b e) -> p b e", e=BLK)
                    t4 = treep.tile([128, nb, 4], F16, tag="t4")
                    nc.vector.tensor_tensor(out=t4[:], in0=sv[:, :, 0:4],
                                            in1=sv[:, :, 4:8],
                                            op=mybir.AluOpType.max)
                    t2 = treep.tile([128, nb, 2], F16, tag="t2")
                    nc.vector.tensor_tensor(out=t2[:], in0=t4[:, :, 0:2],
                                            in1=t4[:, :, 2:4],
                                            op=mybir.AluOpType.max)
                    if j % 2 == 0:
                        bm = bmpool.tile([128, PAIR_BLOCKS], F16, tag="bm")
                    half = slice(0, nb) if j % 2 == 0 else slice(nb, 2 * nb)
                    nc.vector.tensor_tensor(out=bm[:, half], in0=t2[:, :, 0],
                                            in1=t2[:, :, 1],
                                            op=mybir.AluOpType.max)
                    # per-pair (or solo last) top-8 blocks
                    if j % 2 == 1 or j == N_TILES - 1:
                        pi = j // 2
                        width = PAIR_BLOCKS if j % 2 == 1 else nb
                        csl = slice(pi * 8, (pi + 1) * 8)
                        nc.vector.max(cand_v[:, csl], bm[:, 0:width])
                        nc.vector.max_index(cand_i[:, csl], cand_v[:, csl],
                                            bm[:, 0:width])

                # global block gids (fp32; < 2^24 so exact)
                cand_g = candp.tile([128, N_CAND], F32, tag="cg")
                nc.vector.tensor_tensor(
                    out=cand_g[:],
                    in0=cand_i[:].rearrange("p (a b) -> p a b", b=8),
                    in1=ioff[:],
                    op=mybir.AluOpType.add)

                # global top-12 blocks of the 104 candidates
                m8a = sp.tile([128, 8], F16, tag="m8a")
                nc.vector.max(m8a[:], cand_v[:])
                wpos = sp.tile([128, 16], U32, tag="wpos")
                nc.vector.max_index(wpos[:, 0:8], m8a[:], cand_v[:])
                cv2 = candp.tile([128, N_CAND], F16, tag="cv2")
                nc.vector.match_replace(cv2[:], m8a[:], cand_v[:], -60000.0)
                m8b = sp.tile([128, 8], F16, tag="m8b")
                nc.vector.max(m8b[:], cv2[:])
                nc.vector.max_index(wpos[:, 8:16], m8b[:], cv2[:])

                # extract the 12 winner block gids, then gather their 8-row
                # slabs [8 x (embed|norm|pad)] from DRAM
                wgid_f = sp.tile([128, T_WIN], F32, tag="wgidf")
                scratch = candp.tile([128, N_CAND], F32, tag="scr")
                gath = gathp.tile([128, T_WIN, BLK * EW], F32, tag="gath")
                rowoff = sp.tile([128, T_WIN], F32, tag="rowoff")
                rowoff_u = sp.tile([128, T_WIN], U32, tag="rowoffu")
                for k in range(T_WIN):
                    nc.vector.scalar_tensor_tensor(
                        out=scratch[:], in0=iota_cand[:],
                        scalar=wpos[:, k:k + 1], in1=cand_g[:],
                        op0=mybir.AluOpType.is_equal,
                        op1=mybir.AluOpType.mult,
                        accum_out=wgid_f[:, k:k + 1])
                nc.vector.tensor_scalar(out=rowoff[:], in0=wgid_f[:],
                                        scalar1=float(BLK), scalar2=float(h * N_PAD),
                                        op0=mybir.AluOpType.mult,
                                        op1=mybir.AluOpType.add)
                nc.vector.tensor_copy(rowoff_u[:], rowoff[:])
                for k in range(T_WIN):
                    nc.gpsimd.indirect_dma_start(
                        out=gath[:, k], out_offset=None,
                        in_=embn_d[:],
                        in_offset=IndirectOffsetOnAxis(ap=rowoff_u[:, k:k + 1],
                                                       axis=0))

                # exact fp32 recompute for all 96 candidates:
                # s = 2 q.e - (qn + en)  (== -d^2), u = e.f
                ge = gath[:].rearrange("p t (r e) -> p t r e", e=EW)
                prod = prodp.tile([128, T_WIN, BLK, DIM], F32, tag="prod")
                nc.vector.tensor_tensor(
                    out=prod[:], in0=ge[:, :, :, 0:DIM],
                    in1=qv_s.rearrange("p (a b d) -> p a b d", a=1, b=1
                                       ).to_broadcast((128, T_WIN, BLK, DIM)),
                    op=mybir.AluOpType.mult)
                dot = sp.tile([128, NCND], F32, tag="dot")
                nc.vector.tensor_reduce(
                    dot[:].rearrange("p (t r) -> p t r", r=BLK),
                    prod[:], axis=mybir.AxisListType.X,
                    op=mybir.AluOpType.add)
                t96 = sp.tile([128, NCND], F32, tag="t96")
                nc.vector.tensor_scalar(
                    out=t96[:].rearrange("p (t r) -> p t r", r=BLK),
                    in0=ge[:, :, :, DIM], scalar1=qn_s, scalar2=None,
                    op0=mybir.AluOpType.add)
                s96 = s_all[:, m]
                nc.vector.scalar_tensor_tensor(
                    out=s96, in0=dot[:], scalar=2.0, in1=t96[:],
                    op0=mybir.AluOpType.mult, op1=mybir.AluOpType.subtract)

                prodf = prodp.tile([128, T_WIN, BLK, DIM], F32, tag="prod")
                nc.vector.tensor_tensor(
                    out=prodf[:], in0=ge[:, :, :, 0:DIM],
                    in1=fv_s.rearrange("p (a b d) -> p a b d", a=1, b=1
                                       ).to_broadcast((128, T_WIN, BLK, DIM)),
                    op=mybir.AluOpType.mult)
                nc.vector.tensor_reduce(
                    u_all[:, m].rearrange("p (t r) -> p t r", r=BLK),
                    prodf[:], axis=mybir.AxisListType.X,
                    op=mybir.AluOpType.add)

                # exact top-9 of 96, drop rank-1 (self) -> winner mask
                m1 = sp.tile([128, 1], F32, tag="m1")
                nc.vector.tensor_reduce(m1[:], s96, axis=mybir.AxisListType.X,
                                        op=mybir.AluOpType.max)
                m1x8 = sp.tile([128, 8], F32, tag="m1x8")
                nc.vector.tensor_copy(m1x8[:], neg_inf8[:])
                nc.vector.tensor_copy(m1x8[:, 0:1], m1[:])
                srep = sp.tile([128, NCND], F32, tag="srep")
                nc.vector.match_replace(srep[:], m1x8[:], s96, -1e30)
                w8 = sp.tile([128, 8], F32, tag="w8")
                nc.vector.max(w8[:], srep[:])
                srep2 = sp.tile([128, NCND], F32, tag="srep2")
                nc.vector.match_replace(srep2[:], w8[:], srep[:], 1e30)
                mask96 = mask_all[:, m]
                nc.vector.tensor_scalar(out=mask96, in0=srep2[:],
                                        scalar1=1e29, scalar2=None,
                                        op0=mybir.AluOpType.is_ge)

                # debug: (elem gid + 1) * mask so the test can recover winners
                gid96 = sp.tile([128, T_WIN, BLK], F32, tag="gid96")
                nc.vector.scalar_tensor_tensor(
                    out=gid96[:],
                    in0=wgid_f[:].rearrange("p (t o) -> p t o", o=1
                                            ).to_broadcast((128, T_WIN, BLK)),
                    scalar=float(BLK),
                    in1=iota8f[:].rearrange("p (o b) -> p o b", o=1
                                            ).to_broadcast((128, T_WIN, BLK)),
                    op0=mybir.AluOpType.mult, op1=mybir.AluOpType.add)
                gdbg = sp.tile([128, NCND], F32, tag="gdbg")
                nc.vector.scalar_tensor_tensor(
                    out=gdbg[:],
                    in0=gid96[:].rearrange("p t b -> p (t b)"),
                    scalar=1.0, in1=mask96,
                    op0=mybir.AluOpType.add, op1=mybir.AluOpType.mult)
                gdbg_u = sp.tile([128, NCND], U32, tag="gdbgu")
                nc.vector.tensor_copy(gdbg_u[:], gdbg[:])
                nc.sync.dma_start(out=dbg_gid_d[m], in_=gdbg_u[:])
                nc.sync.dma_start(out=dbg_s_d[m], in_=w8[:])

            # batched epilogue: one sqrt/exp table load for all 8 m-tiles
            s_c = accp.tile([128, M_TILES, NCND], F32, tag="s_c")
            nc.vector.tensor_scalar(out=s_c[:], in0=s_all[:], scalar1=0.0,
                                    scalar2=None, op0=mybir.AluOpType.min)
            dist = accp.tile([128, M_TILES, NCND], F32, tag="dist")
            nc.scalar.activation(dist[:], s_c[:],
                                 mybir.ActivationFunctionType.Sqrt,
                                 bias=0.0, scale=-1.0)
            wexp = accp.tile([128, M_TILES, NCND], F32, tag="wexp")
            nc.scalar.activation(wexp[:], dist[:],
                                 mybir.ActivationFunctionType.Exp,
                                 bias=1.0, scale=-1.0)
            wm = accp.tile([128, M_TILES, NCND], F32, tag="wm")
            nc.vector.tensor_tensor(out=wm[:], in0=wexp[:], in1=mask_all[:],
                                    op=mybir.AluOpType.mult)
            numneg = accp.tile([128, M_TILES], F32, tag="numneg")
            scrap = sp.tile([128, NCND], F32, tag="scrap")
            for m in range(M_TILES):
                nc.vector.scalar_tensor_tensor(
                    out=scrap[:], in0=u_all[:, m], scalar=qf_all[:, m:m + 1],
                    in1=wm[:, m],
                    op0=mybir.AluOpType.subtract, op1=mybir.AluOpType.mult,
                    accum_out=numneg[:, m:m + 1])
            wsum = accp.tile([128, M_TILES], F32, tag="wsum")
            nc.vector.tensor_reduce(wsum[:], wm[:], axis=mybir.AxisListType.X,
                                    op=mybir.AluOpType.add)

            # combine heads: pred = sigmoid(mean_h num_h / den_h)
            nsum2 = sp.tile([128, N_HEADS], F32, tag="nsum2")
            nc.vector.tensor_reduce(
                nsum2[:], numneg[:].rearrange("p (h e) -> p h e", e=2),
                axis=mybir.AxisListType.X, op=mybir.AluOpType.add)
            den = sp.tile([128, N_HEADS], F32, tag="den")
            nc.vector.tensor_reduce(
                den[:], wsum[:].rearrange("p (h e) -> p h e", e=2),
                axis=mybir.AxisListType.X, op=mybir.AluOpType.add)
            den8 = sp.tile([128, N_HEADS], F32, tag="den8")
            nc.vector.tensor_scalar(out=den8[:], in0=den[:],
                                    scalar1=float(N_SENT), scalar2=None,
                                    op0=mybir.AluOpType.add)
            rden = sp.tile([128, N_HEADS], F32, tag="rden")
            nc.vector.reciprocal(rden[:], den8[:])
            ratio = sp.tile([128, N_HEADS], F32, tag="ratio")
            nc.vector.tensor_tensor(out=ratio[:], in0=nsum2[:], in1=rden[:],
                                    op=mybir.AluOpType.mult)
            ssum = sp.tile([128, 1], F32, tag="ssum")
            nc.vector.tensor_reduce(ssum[:], ratio[:], axis=mybir.AxisListType.X,
                                    op=mybir.AluOpType.add)
            preds_s = sp.tile([128, 1], F32, tag="preds")
            nc.scalar.activation(preds_s[:], ssum[:],
                                 mybir.ActivationFunctionType.Sigmoid,
                                 bias=0.0, scale=-1.0 / N_HEADS)
            nc.sync.dma_start(out=preds_d[:], in_=preds_s[:])

    return nc


def _prep_inputs(embeds, field, edges):
    """Host-side layout prep + per-core sharding."""
    embeds = np.asarray(embeds, dtype=np.float32)
    field = np.asarray(field, dtype=np.float32)
    edges = np.asarray(edges)

    en = np.sum(np.square(embeds), axis=-1, dtype=np.float32)
    rhs_aug = np.empty((N_HEADS, KC, N_PAD), dtype=np.float32)
    rhs_aug[:, :DIM, :N_NODES] = embeds.transpose(0, 2, 1)
    rhs_aug[:, DIM, :N_NODES] = en
    rhs_aug[:, DIM + 1, :] = -1.0
    rhs_aug[:, :DIM, N_NODES:] = 0.0
    rhs_aug[:, DIM, N_NODES:] = PAD_EN

    embn = np.zeros((N_HEADS * N_PAD, EW), dtype=np.float32)
    ev = embn.reshape(N_HEADS, N_PAD, EW)
    ev[:, :N_NODES, :DIM] = embeds
    ev[:, :N_NODES, DIM] = en
    ev[:, N_NODES:, DIM] = PAD_EN

    in_maps = []
    for c in range(N_CORES):
        sl = slice(c * EDGES_PER_CORE, (c + 1) * EDGES_PER_CORE)
        qpack = np.zeros((M_TILES, KC, 128), dtype=np.float32)
        aux = np.zeros((M_TILES, 128, 2 + 2 * DIM), dtype=np.float32)
        for m in range(M_TILES):
            h, e = m // 2, m % 2
            nodes = edges[e, sl]
            q = embeds[h, nodes]                      # (128, 32)
            f = field[h, nodes]                       # (128, 32)
            qn = np.einsum('bd,bd->b', q, q)
            qpack[m, :DIM] = (2.0 * q).T
            qpack[m, DIM] = -1.0
            qpack[m, DIM + 1] = qn
            aux[m, :, 0] = qn
            aux[m, :, 1] = np.einsum('bd,bd->b', q, f)
            aux[m, :, 2:2 + DIM] = q
            aux[m, :, 2 + DIM:] = f
        in_maps.append({
            "rhs_aug": rhs_aug, "embn": embn,
            "qpack": qpack, "aux": aux,
        })
    return in_maps


def kernel(embeds, field, edges):
    from concourse.bass_utils import run_bass_kernel_spmd

    nc = _build_program()
    nc.finalize()
    in_maps = _prep_inputs(embeds, field, edges)
    core_ids = list(range(N_CORES))
    trace = bool(os.environ.get("KNN_TRACE"))
    tmpdir = os.environ.get("KNN_TRACE_DIR") or None
    out = run_bass_kernel_spmd(nc, in_maps, core_ids, trace=trace,
                               tmpdir=tmpdir)
    LAST["results"] = out
    preds = np.concatenate(
        [out.results[c]["preds"][:, 0] for c in range(N_CORES)])
    return preds.astype(np.float32)


# revision 3
# speedup vs baseline: 1.4700x; 1.4700x over previous
"""Trainium2 Bass kernel for nn_MAD_72679436582977 (retrieval_knn).

For each edge endpoint (src/tgt of 1024 edges) and each of 4 heads: find the
8 nearest neighbors (excluding self) among 50000 nodes in a 32-d embedding
space, logits (q - e_k).f_q, dist |q - e_k|, softmax(1 - dist) over
16 neighbors + 8 sentinels, sigmoid of head-mean weighted sum.

Strategy: data-parallel over edges across 8 cores (128 edges/core, SPMD, no
collectives). Per core, per (head, endpoint) tile of 128 rows:
  - distance GEMM in float32r with K=34 (32 dims + |e|^2 row + |q|^2 row) so
    PSUM holds -d^2 directly; relevant values cluster near 0 where fp16 ULP
    is ~1e-3 (vs rank gaps ~5e-2) -> safe 16-bit candidate scans;
  - ACT (idle otherwise) converts PSUM fp32 -> SBUF fp16 (one full pass);
  - DVE block-max fold tree on fp16 at 2x rate: 2048 -> 256 block maxes of
    8 contiguous nodes (512+256+256 cycles vs 2x2168 for direct max8+index);
  - per 4096-node pair: max8 + find_index8 over 512 block maxes only;
  - top-9 elements provably live in the top-9 blocks by block max; take the
    global top-12 blocks of the 104 candidates (simulated worst-case output
    error 9e-3 incl. fp16 ties and f32r noise, vs 2e-2 budget);
  - indirect-DMA gather of the 12 winning blocks (8 contiguous rows of
    [embed|norm|pad] each) and exact fp32 recompute of s = 2 q.e - (qn+en)
    and u = e.f for all 96 candidates; exact top-9, drop rank-1 (self);
  - epilogue batched across all 8 tiles (single sqrt/exp ACT table loads):
    dist = sqrt(-s), weights exp(1-dist), softmax-ratio with sentinel mass,
    head mean, sigmoid.

Host only shards edges, lays out inputs, and concatenates the 8 per-core
outputs.
"""
import os
import sys

sys.path.insert(0, "/opt/trn_rl_repo")

import numpy as np

import concourse.bass as bass
import concourse.bacc as bacc
import concourse.mybir as mybir
from concourse import tile
from concourse.bass import IndirectOffsetOnAxis

F32 = mybir.dt.float32
F32R = mybir.dt.float32r
F16 = mybir.dt.float16
U32 = mybir.dt.uint32

N_HEADS = 4
N_NODES = 50000
DIM = 32
N_BATCH = 1024
N_SENT = 8
N_CORES = 8

EDGES_PER_CORE = N_BATCH // N_CORES          # 128
TILE_W = 2048                                 # PSUM tile (4 banks)
N_TILES = 25                                  # 24 full + 1 half
LAST_W = 1024
N_PAD = TILE_W * (N_TILES - 1) + LAST_W       # 50176
M_TILES = N_HEADS * 2                         # (head, src/tgt) x 128 rows
KC = DIM + 2                                  # 32 dims + en row + qn row
BLK = 8                                       # nodes per block
N_BLOCKS = N_PAD // BLK                       # 6272
N_PAIRS = 12                                  # pairs of 2048-tiles (4096 nodes)
PAIR_BLOCKS = 2 * (TILE_W // BLK)             # 512
N_CAND = N_PAIRS * 8 + 8                      # 104 candidate blocks
T_WIN = 12                                    # winning blocks kept
NCND = T_WIN * BLK                            # 96 candidate nodes
EW = DIM + 2                                  # embn row: embed|norm|pad
PAD_EN = 60000.0

LAST = {}


def _build_program():
    nc = bacc.Bacc(None, num_swdge_queues=2)

    rhs_d = nc.dram_tensor("rhs_aug", [N_HEADS, KC, N_PAD], F32R,
                           kind="ExternalInput")
    embn_d = nc.dram_tensor("embn", [N_HEADS * N_PAD, EW], F32,
                            kind="ExternalInput")
    qpack_d = nc.dram_tensor("qpack", [M_TILES, KC, 128], F32R,
                             kind="ExternalInput")
    aux_d = nc.dram_tensor("aux", [M_TILES, 128, 2 + 2 * DIM], F32,
                           kind="ExternalInput")

    preds_d = nc.dram_tensor("preds", [128, 1], F32, kind="ExternalOutput")
    dbg_gid_d = nc.dram_tensor("dbg_gid", [M_TILES, 128, NCND], U32,
                               kind="ExternalOutput")
    dbg_s_d = nc.dram_tensor("dbg_s", [M_TILES, 128, 8], F32,
                             kind="ExternalOutput")

    with tile.TileContext(nc) as tc:
        with tc.tile_pool(name="const", bufs=1) as cpool, \
             tc.tile_pool(name="qp", bufs=2) as qpool, \
             tc.tile_pool(name="rhs", bufs=3) as rpool, \
             tc.tile_pool(name="s16", bufs=3) as s16p, \
             tc.tile_pool(name="tree", bufs=2) as treep, \
             tc.tile_pool(name="bm", bufs=2) as bmpool, \
             tc.tile_pool(name="cand", bufs=2) as candp, \
             tc.tile_pool(name="gath", bufs=2) as gathp, \
             tc.tile_pool(name="prod", bufs=2) as prodp, \
             tc.tile_pool(name="small", bufs=3) as sp, \
             tc.tile_pool(name="acc", bufs=1) as accp, \
             tc.tile_pool(name="ps", bufs=2, space="PSUM") as psp:

            # constants
            iota_cand = cpool.tile([128, N_CAND], U32, tag="iota_cand")
            nc.gpsimd.iota(iota_cand[:], pattern=[[1, N_CAND]], base=0,
                           channel_multiplier=0)
            iota8f = cpool.tile([128, 8], F32, tag="iota8f")
            nc.gpsimd.iota(iota8f[:], pattern=[[1, 8]], base=0,
                           channel_multiplier=0,
                           allow_small_or_imprecise_dtypes=True)
            # per-pair block-gid offsets: gid = pair*512 + within (13th = solo)
            ioff = cpool.tile([128, N_PAIRS + 1, 8], F32, tag="ioff")
            nc.gpsimd.iota(ioff[:], pattern=[[PAIR_BLOCKS, N_PAIRS + 1], [0, 8]],
                           base=0, channel_multiplier=0,
                           allow_small_or_imprecise_dtypes=True)
            neg_inf8 = cpool.tile([128, 8], F32, tag="neg_inf8")
            nc.vector.memset(neg_inf8[:], -1e30)

            # retained per-m state for the batched epilogue
            s_all = accp.tile([128, M_TILES, NCND], F32, tag="s_all")
            u_all = accp.tile([128, M_TILES, NCND], F32, tag="u_all")
            mask_all = accp.tile([128, M_TILES, NCND], F32, tag="mask_all")
            qf_all = accp.tile([128, M_TILES], F32, tag="qf_all")

            for m in range(M_TILES):
                h = m // 2
                q_s = qpool.tile([KC, 128], F32R, tag="q")
                nc.sync.dma_start(out=q_s[:], in_=qpack_d[m])
                aux_s = qpool.tile([128, 2 + 2 * DIM], F32, tag="aux")
                nc.sync.dma_start(out=aux_s[:], in_=aux_d[m])
                qn_s = aux_s[:, 0:1]
                qf_s = aux_s[:, 1:2]
                qv_s = aux_s[:, 2:2 + DIM]
                fv_s = aux_s[:, 2 + DIM:2 + 2 * DIM]
                nc.vector.tensor_copy(qf_all[:, m:m + 1], qf_s)

                cand_v = candp.tile([128, N_CAND], F16, tag="cv")
                cand_i = candp.tile([128, N_CAND], U32, tag="ci")

                bm = None
                for j in range(N_TILES):
                    w = TILE_W if j < N_TILES - 1 else LAST_W
                    nb = w // BLK
                    rhs_s = rpool.tile([KC, w], F32R, tag="rhs")
                    # split 32+2 rows: a 34-row DMA sprays over only 2 DMA
                    # engines (largest divisor <= 16), a 32-row one over 16
                    nc.sync.dma_start(
                        out=rhs_s[0:DIM],
                        in_=rhs_d[h, 0:DIM, j * TILE_W:j * TILE_W + w])
                    nc.sync.dma_start(
                        out=rhs_s[DIM:KC],
                        in_=rhs_d[h, DIM:KC, j * TILE_W:j * TILE_W + w])
                    psum = psp.tile([128, w], F32, tag="ps")
                    for b in range(w // 512):
                        sl = slice(b * 512, (b + 1) * 512)
                        nc.tensor.matmul(psum[:, sl], q_s[:], rhs_s[:, sl],
                                         start=True, stop=True)
                    # fp32 -> fp16 full pass on the (otherwise idle) scalar eng
                    s16 = s16p.tile([128, w], F16, tag="s16")
                    nc.scalar.activation(s16[:], psum[:],
                                         mybir.ActivationFunctionType.Copy,
                                         bias=0.0, scale=1.0)
                    # fold tree to per-block (8) maxes; f1/f2 run at 2x (fp16)
                    sv = s16[:].rearrange("p (b e) -> p b e", e=BLK)
                    t4 = treep.tile([128, nb, 4], F16, tag="t4")
                    nc.vector.tensor_tensor(out=t4[:], in0=sv[:, :, 0:4],
                                            in1=sv[:, :, 4:8],
                                            op=mybir.AluOpType.max)
                    t2 = treep.tile([128, nb, 2], F16, tag="t2")
                    nc.vector.tensor_tensor(out=t2[:], in0=t4[:, :, 0:2],
                                            in1=t4[:, :, 2:4],
                                            op=mybir.AluOpType.max)
                    if j % 2 == 0:
                        bm = bmpool.tile([128, PAIR_BLOCKS], F16, tag="bm")
                    half = slice(0, nb) if j % 2 == 0 else slice(nb, 2 * nb)
                    nc.vector.tensor_tensor(out=bm[:, half], in0=t2[:, :, 0],
                                            in1=t2[:, :, 1],
                                            op=mybir.AluOpType.max)
                    # per-pair (or solo last) top-8 blocks
                    if j % 2 == 1 or j == N_TILES - 1:
                        pi = j // 2
                        width = PAIR_BLOCKS if j % 2 == 1 else nb
                        csl = slice(pi * 8, (pi + 1) * 8)
                        nc.vector.max(cand_v[:, csl], bm[:, 0:width])
                        nc.vector.max_index(cand_i[:, csl], cand_v[:, csl],
                                            bm[:, 0:width])

                # global block gids (fp32; < 2^24 so exact)
                cand_g = candp.tile([128, N_CAND], F32, tag="cg")
                nc.vector.tensor_tensor(
                    out=cand_g[:],
                    in0=cand_i[:].rearrange("p (a b) -> p a b", b=8),
                    in1=ioff[:],
                    op=mybir.AluOpType.add)

                # global top-12 blocks of the 104 candidates
                m8a = sp.tile([128, 8], F16, tag="m8a")
                nc.vector.max(m8a[:], cand_v[:])
                wpos = sp.tile([128, 16], U32, tag="wpos")
                nc.vector.max_index(wpos[:, 0:8], m8a[:], cand_v[:])
                cv2 = candp.tile([128, N_CAND], F16, tag="cv2")
                nc.vector.match_replace(cv2[:], m8a[:], cand_v[:], -60000.0)
                m8b = sp.tile([128, 8], F16, tag="m8b")
                nc.vector.max(m8b[:], cv2[:])
                nc.vector.max_index(wpos[:, 8:16], m8b[:], cv2[:])

                # extract the 12 winner block gids, then gather their 8-row
                # slabs [8 x (embed|norm|pad)] from DRAM
                wgid_f = sp.tile([128, T_WIN], F32, tag="wgidf")
                scratch = candp.tile([128, N_CAND], F32, tag="scr")
                gath = gathp.tile([128, T_WIN, BLK * EW], F32, tag="gath")
                rowoff = sp.tile([128, T_WIN], F32, tag="rowoff")
                rowoff_u = sp.tile([128, T_WIN], U32, tag="rowoffu")
                for k in range(T_WIN):
                    nc.vector.scalar_tensor_tensor(
                        out=scratch[:], in0=iota_cand[:],
                        scalar=wpos[:, k:k + 1], in1=cand_g[:],
                        op0=mybir.AluOpType.is_equal,
                        op1=mybir.AluOpType.mult,
                        accum_out=wgid_f[:, k:k + 1])
                nc.vector.tensor_scalar(out=rowoff[:], in0=wgid_f[:],
                                        scalar1=float(BLK), scalar2=float(h * N_PAD),
                                        op0=mybir.AluOpType.mult,
                                        op1=mybir.AluOpType.add)
                nc.vector.tensor_copy(rowoff_u[:], rowoff[:])
                for k in range(T_WIN):
                    nc.gpsimd.indirect_dma_start(
                        out=gath[:, k], out_offset=None,
                        in_=embn_d[:],
                        in_offset=IndirectOffsetOnAxis(ap=rowoff_u[:, k:k + 1],
                                                       axis=0))

                # exact fp32 recompute for all 96 candidates:
                # s = 2 q.e - (qn + en)  (== -d^2), u = e.f
                ge = gath[:].rearrange("p t (r e) -> p t r e", e=EW)
                prod = prodp.tile([128, T_WIN, BLK, DIM], F32, tag="prod")
                nc.vector.tensor_tensor(
                    out=prod[:], in0=ge[:, :, :, 0:DIM],
                    in1=qv_s.rearrange("p (a b d) -> p a b d", a=1, b=1
                                       ).to_broadcast((128, T_WIN, BLK, DIM)),
                    op=mybir.AluOpType.mult)
                dot = sp.tile([128, NCND], F32, tag="dot")
                nc.vector.tensor_reduce(
                    dot[:].rearrange("p (t r) -> p t r", r=BLK),
                    prod[:], axis=mybir.AxisListType.X,
                    op=mybir.AluOpType.add)
                t96 = sp.tile([128, NCND], F32, tag="t96")
                nc.vector.tensor_scalar(
                    out=t96[:].rearrange("p (t r) -> p t r", r=BLK),
                    in0=ge[:, :, :, DIM], scalar1=qn_s, scalar2=None,
                    op0=mybir.AluOpType.add)
                s96 = s_all[:, m]
                nc.vector.scalar_tensor_tensor(
                    out=s96, in0=dot[:], scalar=2.0, in1=t96[:],
                    op0=mybir.AluOpType.mult, op1=mybir.AluOpType.subtract)

                prodf = prodp.tile([128, T_WIN, BLK, DIM], F32, tag="prod")
                nc.vector.tensor_tensor(
                    out=prodf[:], in0=ge[:, :, :, 0:DIM],
                    in1=fv_s.rearrange("p (a b d) -> p a b d", a=1, b=1
                                       ).to_broadcast((128, T_WIN, BLK, DIM)),
                    op=mybir.AluOpType.mult)
                nc.vector.tensor_reduce(
                    u_all[:, m].rearrange("p (t r) -> p t r", r=BLK),
                    prodf[:], axis=mybir.AxisListType.X,
                    op=mybir.AluOpType.add)

                # exact top-9 of 96, drop rank-1 (self) -> winner mask
                m1 = sp.tile([128, 1], F32, tag="m1")
                nc.vector.tensor_reduce(m1[:], s96, axis=mybir.AxisListType.X,
                                        op=mybir.AluOpType.max)
                m1x8 = sp.tile([128, 8], F32, tag="m1x8")
                nc.vector.tensor_copy(m1x8[:], neg_inf8[:])
                nc.vector.tensor_copy(m1x8[:, 0:1], m1[:])
                srep = sp.tile([128, NCND], F32, tag="srep")
                nc.vector.match_replace(srep[:], m1x8[:], s96, -1e30)
                w8 = sp.tile([128, 8], F32, tag="w8")
                nc.vector.max(w8[:], srep[:])
                srep2 = sp.tile([128, NCND], F32, tag="srep2")
                nc.vector.match_replace(srep2[:], w8[:], srep[:], 1e30)
                mask96 = mask_all[:, m]
                nc.vector.tensor_scalar(out=mask96, in0=srep2[:],
                                        scalar1=1e29, scalar2=None,
                                        op0=mybir.AluOpType.is_ge)

                # debug: (elem gid + 1) * mask so the test can recover winners
                gid96 = sp.tile([128, T_WIN, BLK], F32, tag="gid96")
                nc.vector.scalar_tensor_tensor(
                    out=gid96[:],
                    in0=wgid_f[:].rearrange("p (t o) -> p t o", o=1
                                            ).to_broadcast((128, T_WIN, BLK)),
                    scalar=float(BLK),
                    in1=iota8f[:].rearrange("p (o b) -> p o b", o=1
                                            ).to_broadcast((128, T_WIN, BLK)),
                    op0=mybir.AluOpType.mult, op1=mybir.AluOpType.add)
                gdbg = sp.tile([128, NCND], F32, tag="gdbg")
                nc.vector.scalar_tensor_tensor(
                    out=gdbg[:],
                    in0=gid96[:].rearrange("p t b -> p (t b)"),
                    scalar=1.0, in1=mask96,
                    op0=mybir.AluOpType.add, op1=mybir.AluOpType.mult)
                gdbg_u = sp.tile([128, NCND], U32, tag="gdbgu")
                nc.vector.tensor_copy(gdbg_u[:], gdbg[:])
                nc.sync.dma_start(out=dbg_gid_d[m], in_=gdbg_u[:])
                nc.sync.dma_start(out=dbg_s_d[m], in_=w8[:])

            # batched epilogue: one sqrt/exp table load for all 8 m-tiles
            s_c = accp.tile([128, M_TILES, NCND], F32, tag="s_c")
            nc.vector.tensor_scalar(out=s_c[:], in0=s_all[:], scalar1=0.0,
                                    scalar2=None, op0=mybir.AluOpType.min)
            dist = accp.tile([128, M_TILES, NCND], F32, tag="dist")
            nc.scalar.activation(dist[:], s_c[:],
                                 mybir.ActivationFunctionType.Sqrt,
                                 bias=0.0, scale=-1.0)
            wexp = accp.tile([128, M_TILES, NCND], F32, tag="wexp")
            nc.scalar.activation(wexp[:], dist[:],
                                 mybir.ActivationFunctionType.Exp,
                                 bias=1.0, scale=-1.0)
            wm = accp.tile([128, M_TILES, NCND], F32, tag="wm")
            nc.vector.tensor_tensor(out=wm[:], in0=wexp[:], in1=mask_all[:],
                                    op=mybir.AluOpType.mult)
            numneg = accp.tile([128, M_TILES], F32, tag="numneg")
            scrap = sp.tile([128, NCND], F32, tag="scrap")
            for m in range(M_TILES):
                nc.vector.scalar_tensor_tensor(
                    out=scrap[:], in0=u_all[:, m], scalar=qf_all[:, m:m + 1],
                    in1=wm[:, m],
                    op0=mybir.AluOpType.subtract, op1=mybir.AluOpType.mult,
                    accum_out=numneg[:, m:m + 1])
            wsum = accp.tile([128, M_TILES], F32, tag="wsum")
            nc.vector.tensor_reduce(wsum[:], wm[:], axis=mybir.AxisListType.X,
                                    op=mybir.AluOpType.add)

            # combine heads: pred = sigmoid(mean_h num_h / den_h)
            nsum2 = sp.tile([128, N_HEADS], F32, tag="nsum2")
            nc.vector.tensor_reduce(
                nsum2[:], numneg[:].rearrange("p (h e) -> p h e", e=2),
                axis=mybir.AxisListType.X, op=mybir.AluOpType.add)
            den = sp.tile([128, N_HEADS], F32, tag="den")
            nc.vector.tensor_reduce(
                den[:], wsum[:].rearrange("p (h e) -> p h e", e=2),
                axis=mybir.AxisListType.X, op=mybir.AluOpType.add)
            den8 = sp.tile([128, N_HEADS], F32, tag="den8")
            nc.vector.tensor_scalar(out=den8[:], in0=den[:],
                                    scalar1=float(N_SENT), scalar2=None,
                                    op0=mybir.AluOpType.add)
            rden = sp.tile([128, N_HEADS], F32, tag="rden")
            nc.vector.reciprocal(rden[:], den8[:])
            ratio = sp.tile([128, N_HEADS], F32, tag="ratio")
            nc.vector.tensor_tensor(out=ratio[:], in0=nsum2[:], in1=rden[:],
                                    op=mybir.AluOpType.mult)
            ssum = sp.tile([128, 1], F32, tag="ssum")
            nc.vector.tensor_reduce(ssum[:], ratio[:], axis=mybir.AxisListType.X,
                                    op=mybir.AluOpType.add)
            preds_s = sp.tile([128, 1], F32, tag="preds")
            nc.scalar.activation(preds_s[:], ssum[:],
                                 mybir.ActivationFunctionType.Sigmoid,
                                 bias=0.0, scale=-1.0 / N_HEADS)
            nc.sync.dma_start(out=preds_d[:], in_=preds_s[:])

    return nc


def _prep_inputs(embeds, field, edges):
    """Host-side layout prep + per-core sharding."""
    embeds = np.asarray(embeds, dtype=np.float32)
    field = np.asarray(field, dtype=np.float32)
    edges = np.asarray(edges)

    en = np.sum(np.square(embeds), axis=-1, dtype=np.float32)
    rhs_aug = np.empty((N_HEADS, KC, N_PAD), dtype=np.float32)
    rhs_aug[:, :DIM, :N_NODES] = embeds.transpose(0, 2, 1)
    rhs_aug[:, DIM, :N_NODES] = en
    rhs_aug[:, DIM + 1, :] = -1.0
    rhs_aug[:, :DIM, N_NODES:] = 0.0
    rhs_aug[:, DIM, N_NODES:] = PAD_EN

    embn = np.zeros((N_HEADS * N_PAD, EW), dtype=np.float32)
    ev = embn.reshape(N_HEADS, N_PAD, EW)
    ev[:, :N_NODES, :DIM] = embeds
    ev[:, :N_NODES, DIM] = en
    ev[:, N_NODES:, DIM] = PAD_EN

    in_maps = []
    for c in range(N_CORES):
        sl = slice(c * EDGES_PER_CORE, (c + 1) * EDGES_PER_CORE)
        qpack = np.zeros((M_TILES, KC, 128), dtype=np.float32)
        aux = np.zeros((M_TILES, 128, 2 + 2 * DIM), dtype=np.float32)
        for m in range(M_TILES):
            h, e = m // 2, m % 2
            nodes = edges[e, sl]
            q = embeds[h, nodes]                      # (128, 32)
            f = field[h, nodes]                       # (128, 32)
            qn = np.einsum('bd,bd->b', q, q)
            qpack[m, :DIM] = (2.0 * q).T
            qpack[m, DIM] = -1.0
            qpack[m, DIM + 1] = qn
            aux[m, :, 0] = qn
            aux[m, :, 1] = np.einsum('bd,bd->b', q, f)
            aux[m, :, 2:2 + DIM] = q
            aux[m, :, 2 + DIM:] = f
        in_maps.append({
            "rhs_aug": rhs_aug, "embn": embn,
            "qpack": qpack, "aux": aux,
        })
    return in_maps


def kernel(embeds, field, edges):
    from concourse.bass_utils import run_bass_kernel_spmd

    nc = _build_program()
    nc.finalize()
    in_maps = _prep_inputs(embeds, field, edges)
    core_ids = list(range(N_CORES))
    trace = bool(os.environ.get("KNN_TRACE"))
    tmpdir = os.environ.get("KNN_TRACE_DIR") or None
    out = run_bass_kernel_spmd(nc, in_maps, core_ids, trace=trace,
                               tmpdir=tmpdir)
    LAST["results"] = out
    preds = np.concatenate(
        [out.results[c]["preds"][:, 0] for c in range(N_CORES)])
    return preds.astype(np.float32)


# revision 7
# speedup vs baseline: 1.4944x; 1.0166x over previous
"""Trainium2 Bass kernel for nn_MAD_72679436582977 (retrieval_knn).

For each edge endpoint (src/tgt of 1024 edges) and each of 4 heads: find the
8 nearest neighbors (excluding self) among 50000 nodes in a 32-d embedding
space, logits (q - e_k).f_q, dist |q - e_k|, softmax(1 - dist) over
16 neighbors + 8 sentinels, sigmoid of head-mean weighted sum.

Strategy: data-parallel over edges across 8 cores (128 edges/core, SPMD, no
collectives). Per core, per (head, endpoint) tile of 128 rows:
  - distance GEMM in float32r with K=34 (32 dims + |e|^2 row + |q|^2 row) so
    PSUM holds -d^2 directly; relevant values cluster near 0 where fp16 ULP
    is ~1e-3 (vs rank gaps ~5e-2) -> safe 16-bit candidate scans;
  - ACT (idle otherwise) converts PSUM fp32 -> SBUF fp16 (one full pass);
  - DVE block-max fold tree on fp16 at 2x rate: 2048 -> 256 block maxes of
    8 contiguous nodes (512+256+256 cycles vs 2x2168 for direct max8+index);
  - per 4096-node pair: max8 + find_index8 over 512 block maxes only;
  - top-9 elements provably live in the top-9 blocks by block max; take the
    global top-12 blocks of the 104 candidates (simulated worst-case output
    error 9e-3 incl. fp16 ties and f32r noise, vs 2e-2 budget);
  - indirect-DMA gather of the 12 winning blocks (8 contiguous rows of
    [embed|norm|pad] each) and exact fp32 recompute of s = 2 q.e - (qn+en)
    and u = e.f for all 96 candidates; exact top-9, drop rank-1 (self);
  - epilogue batched across all 8 tiles (single sqrt/exp ACT table loads):
    dist = sqrt(-s), weights exp(1-dist), softmax-ratio with sentinel mass,
    head mean, sigmoid.

Host only shards edges, lays out inputs, and concatenates the 8 per-core
outputs.
"""
import os
import sys

sys.path.insert(0, "/opt/trn_rl_repo")

import numpy as np

import concourse.bass as bass
import concourse.bacc as bacc
import concourse.mybir as mybir
from concourse import tile
from concourse.bass import IndirectOffsetOnAxis

F32 = mybir.dt.float32
F32R = mybir.dt.float32r
F16 = mybir.dt.float16
U32 = mybir.dt.uint32

N_HEADS = 4
N_NODES = 50000
DIM = 32
N_BATCH = 1024
N_SENT = 8
N_CORES = 8

EDGES_PER_CORE = N_BATCH // N_CORES          # 128
TILE_W = 2048                                 # PSUM tile (4 banks)
N_TILES = 25                                  # 24 full + 1 half
LAST_W = 1024
N_PAD = TILE_W * (N_TILES - 1) + LAST_W       # 50176
M_TILES = N_HEADS * 2                         # (head, src/tgt) x 128 rows
KC = DIM + 2                                  # 32 dims + en row + qn row
BLK = 8                                       # nodes per block
N_BLOCKS = N_PAD // BLK                       # 6272
N_PAIRS = 12                                  # pairs of 2048-tiles (4096 nodes)
PAIR_BLOCKS = 2 * (TILE_W // BLK)             # 512
N_CAND = N_PAIRS * 8 + 8                      # 104 candidate blocks
T_WIN = 12                                    # winning blocks kept
NCND = T_WIN * BLK                            # 96 candidate nodes
EW = DIM + 2                                  # embn row: embed|norm|pad
PAD_EN = 60000.0

LAST = {}


def _build_program():
    nc = bacc.Bacc(None, num_swdge_queues=2)

    rhs_d = nc.dram_tensor("rhs_aug", [N_HEADS, KC, N_PAD], F32R,
                           kind="ExternalInput")
    embn_d = nc.dram_tensor("embn", [N_HEADS * N_PAD, EW], F32,
                            kind="ExternalInput")
    qpack_d = nc.dram_tensor("qpack", [M_TILES, KC, 128], F32R,
                             kind="ExternalInput")
    aux_d = nc.dram_tensor("aux", [M_TILES, 128, 2 + 2 * DIM], F32,
                           kind="ExternalInput")

    preds_d = nc.dram_tensor("preds", [128, 1], F32, kind="ExternalOutput")
    dbg_gid_d = nc.dram_tensor("dbg_gid", [M_TILES, 128, NCND], U32,
                               kind="ExternalOutput")
    dbg_s_d = nc.dram_tensor("dbg_s", [M_TILES, 128, 8], F32,
                             kind="ExternalOutput")

    with tile.TileContext(nc) as tc:
        with tc.tile_pool(name="const", bufs=1) as cpool, \
             tc.tile_pool(name="qp", bufs=2) as qpool, \
             tc.tile_pool(name="rhs", bufs=3) as rpool, \
             tc.tile_pool(name="s16", bufs=3) as s16p, \
             tc.tile_pool(name="tree", bufs=2) as treep, \
             tc.tile_pool(name="bm", bufs=2) as bmpool, \
             tc.tile_pool(name="cand", bufs=2) as candp, \
             tc.tile_pool(name="gath", bufs=2) as gathp, \
             tc.tile_pool(name="prod", bufs=2) as prodp, \
             tc.tile_pool(name="small", bufs=3) as sp, \
             tc.tile_pool(name="acc", bufs=1) as accp, \
             tc.tile_pool(name="ps", bufs=2, space="PSUM") as psp:

            # constants
            iota_cand = cpool.tile([128, N_CAND], U32, tag="iota_cand")
            nc.gpsimd.iota(iota_cand[:], pattern=[[1, N_CAND]], base=0,
                           channel_multiplier=0)
            iota8f = cpool.tile([128, 8], F32, tag="iota8f")
            nc.gpsimd.iota(iota8f[:], pattern=[[1, 8]], base=0,
                           channel_multiplier=0,
                           allow_small_or_imprecise_dtypes=True)
            # per-pair block-gid offsets: gid = pair*512 + within (13th = solo)
            ioff = cpool.tile([128, N_PAIRS + 1, 8], F32, tag="ioff")
            nc.gpsimd.iota(ioff[:], pattern=[[PAIR_BLOCKS, N_PAIRS + 1], [0, 8]],
                           base=0, channel_multiplier=0,
                           allow_small_or_imprecise_dtypes=True)
            neg_inf8 = cpool.tile([128, 8], F32, tag="neg_inf8")
            nc.vector.memset(neg_inf8[:], -1e30)

            # retained per-m state for the batched epilogue
            s_all = accp.tile([128, M_TILES, NCND], F32, tag="s_all")
            u_all = accp.tile([128, M_TILES, NCND], F32, tag="u_all")
            mask_all = accp.tile([128, M_TILES, NCND], F32, tag="mask_all")
            qf_all = accp.tile([128, M_TILES], F32, tag="qf_all")

            def phase_a(m):
                """Supers, candidate blocks, winner extraction, gathers."""
                h = m // 2
                q_s = qpool.tile([KC, 128], F32R, tag="q")
                nc.sync.dma_start(out=q_s[:], in_=qpack_d[m])
                aux_s = qpool.tile([128, 2 + 2 * DIM], F32, tag="aux")
                nc.sync.dma_start(out=aux_s[:], in_=aux_d[m])
                qn_s = aux_s[:, 0:1]
                qf_s = aux_s[:, 1:2]
                nc.vector.tensor_copy(qf_all[:, m:m + 1], qf_s)

                cand_v = candp.tile([128, N_CAND], F16, tag="cv")
                cand_i = candp.tile([128, N_CAND], U32, tag="ci")

                bm = None
                for j in range(N_TILES):  # noqa: B007
                    w = TILE_W if j < N_TILES - 1 else LAST_W
                    nb = w // BLK
                    rhs_s = rpool.tile([KC, w], F32R, tag="rhs")
                    # split 32+2 rows: a 34-row DMA sprays over only 2 DMA
                    # engines (largest divisor <= 16), a 32-row one over 16
                    nc.sync.dma_start(
                        out=rhs_s[0:DIM],
                        in_=rhs_d[h, 0:DIM, j * TILE_W:j * TILE_W + w])
                    nc.sync.dma_start(
                        out=rhs_s[DIM:KC],
                        in_=rhs_d[h, DIM:KC, j * TILE_W:j * TILE_W + w])
                    psum = psp.tile([128, w], F32, tag="ps")
                    for b in range(w // 512):
                        sl = slice(b * 512, (b + 1) * 512)
                        nc.tensor.matmul(psum[:, sl], q_s[:], rhs_s[:, sl],
                                         start=True, stop=True)
                    # fp32 -> fp16 full pass on the (otherwise idle) scalar eng
                    s16 = s16p.tile([128, w], F16, tag="s16")
                    nc.scalar.activation(s16[:], psum[:],
                                         mybir.ActivationFunctionType.Copy,
                                         bias=0.0, scale=1.0)
                    # fold tree to per-block (8) maxes; f1/f2 run at 2x (fp16)
                    sv = s16[:].rearrange("p (b e) -> p b e", e=BLK)
                    t4 = treep.tile([128, nb, 4], F16, tag="t4")
                    nc.vector.tensor_tensor(out=t4[:], in0=sv[:, :, 0:4],
                                            in1=sv[:, :, 4:8],
                                            op=mybir.AluOpType.max)
                    t2 = treep.tile([128, nb, 2], F16, tag="t2")
                    nc.vector.tensor_tensor(out=t2[:], in0=t4[:, :, 0:2],
                                            in1=t4[:, :, 2:4],
                                            op=mybir.AluOpType.max)
                    if j % 2 == 0:
                        bm = bmpool.tile([128, PAIR_BLOCKS], F16, tag="bm")
                    half = slice(0, nb) if j % 2 == 0 else slice(nb, 2 * nb)
                    nc.vector.tensor_tensor(out=bm[:, half], in0=t2[:, :, 0],
                                            in1=t2[:, :, 1],
                                            op=mybir.AluOpType.max)
                    # per-pair (or solo last) top-8 blocks
                    if j % 2 == 1 or j == N_TILES - 1:
                        pi = j // 2
                        width = PAIR_BLOCKS if j % 2 == 1 else nb
                        csl = slice(pi * 8, (pi + 1) * 8)
                        nc.vector.max(cand_v[:, csl], bm[:, 0:width])
                        nc.vector.max_index(cand_i[:, csl], cand_v[:, csl],
                                            bm[:, 0:width])

                # global block gids (fp32; < 2^24 so exact)
                cand_g = candp.tile([128, N_CAND], F32, tag="cg")
                nc.vector.tensor_tensor(
                    out=cand_g[:],
                    in0=cand_i[:].rearrange("p (a b) -> p a b", b=8),
                    in1=ioff[:],
                    op=mybir.AluOpType.add)

                # global top-12 blocks of the 104 candidates
                m8a = sp.tile([128, 8], F16, tag="m8a")
                nc.vector.max(m8a[:], cand_v[:])
                wpos = sp.tile([128, 16], U32, tag="wpos")
                nc.vector.max_index(wpos[:, 0:8], m8a[:], cand_v[:])
                cv2 = candp.tile([128, N_CAND], F16, tag="cv2")
                nc.vector.match_replace(cv2[:], m8a[:], cand_v[:], -60000.0)
                m8b = sp.tile([128, 8], F16, tag="m8b")
                nc.vector.max(m8b[:], cv2[:])
                nc.vector.max_index(wpos[:, 8:16], m8b[:], cv2[:])

                # extract the 12 winner block gids, then gather their 8-row
                # slabs [8 x (embed|norm|pad)] from DRAM
                wgid_f = sp.tile([128, T_WIN], F32, tag="wgidf")
                scratch = candp.tile([128, N_CAND], F32, tag="scr")
                gath = gathp.tile([128, T_WIN, BLK * EW], F32, tag="gath")
                rowoff = sp.tile([128, T_WIN], F32, tag="rowoff")
                rowoff_u = sp.tile([128, T_WIN], U32, tag="rowoffu")
                for k in range(T_WIN):
                    nc.vector.scalar_tensor_tensor(
                        out=scratch[:], in0=iota_cand[:],
                        scalar=wpos[:, k:k + 1], in1=cand_g[:],
                        op0=mybir.AluOpType.is_equal,
                        op1=mybir.AluOpType.mult,
                        accum_out=wgid_f[:, k:k + 1])
                nc.vector.tensor_scalar(out=rowoff[:], in0=wgid_f[:],
                                        scalar1=float(BLK), scalar2=float(h * N_PAD),
                                        op0=mybir.AluOpType.mult,
                                        op1=mybir.AluOpType.add)
                nc.vector.tensor_copy(rowoff_u[:], rowoff[:])
                for k in range(T_WIN):
                    nc.gpsimd.indirect_dma_start(
                        out=gath[:, k], out_offset=None,
                        in_=embn_d[:],
                        in_offset=IndirectOffsetOnAxis(ap=rowoff_u[:, k:k + 1],
                                                       axis=0))
                return dict(m=m, aux_s=aux_s, wgid_f=wgid_f, gath=gath)

            def phase_b(st):
                """Gather-dependent tail: exact dots, selection, debug.

                Emitted one m behind phase_a so the strict-FIFO DVE queue has
                the next tile's tree work queued while the gathers land.
                """
                m, aux_s, wgid_f, gath = st["m"], st["aux_s"], st["wgid_f"], st["gath"]
                qn_s = aux_s[:, 0:1]
                qv_s = aux_s[:, 2:2 + DIM]
                fv_s = aux_s[:, 2 + DIM:2 + 2 * DIM]
                # exact fp32 recompute for all 96 candidates:
                # s = 2 q.e - (qn + en)  (== -d^2), u = e.f
                ge = gath[:].rearrange("p t (r e) -> p t r e", e=EW)
                prod = prodp.tile([128, T_WIN, BLK, DIM], F32, tag="prod")
                nc.vector.tensor_tensor(
                    out=prod[:], in0=ge[:, :, :, 0:DIM],
                    in1=qv_s.rearrange("p (a b d) -> p a b d", a=1, b=1
                                       ).to_broadcast((128, T_WIN, BLK, DIM)),
                    op=mybir.AluOpType.mult)
                dot = sp.tile([128, NCND], F32, tag="dot")
                nc.vector.tensor_reduce(
                    dot[:].rearrange("p (t r) -> p t r", r=BLK),
                    prod[:], axis=mybir.AxisListType.X,
                    op=mybir.AluOpType.add)
                t96 = sp.tile([128, NCND], F32, tag="t96")
                nc.vector.tensor_scalar(
                    out=t96[:].rearrange("p (t r) -> p t r", r=BLK),
                    in0=ge[:, :, :, DIM], scalar1=qn_s, scalar2=None,
                    op0=mybir.AluOpType.add)
                s96 = s_all[:, m]
                nc.vector.scalar_tensor_tensor(
                    out=s96, in0=dot[:], scalar=2.0, in1=t96[:],
                    op0=mybir.AluOpType.mult, op1=mybir.AluOpType.subtract)

                prodf = prodp.tile([128, T_WIN, BLK, DIM], F32, tag="prod")
                nc.vector.tensor_tensor(
                    out=prodf[:], in0=ge[:, :, :, 0:DIM],
                    in1=fv_s.rearrange("p (a b d) -> p a b d", a=1, b=1
                                       ).to_broadcast((128, T_WIN, BLK, DIM)),
                    op=mybir.AluOpType.mult)
                nc.vector.tensor_reduce(
                    u_all[:, m].rearrange("p (t r) -> p t r", r=BLK),
                    prodf[:], axis=mybir.AxisListType.X,
                    op=mybir.AluOpType.add)

                # exact top-9 of 96, drop rank-1 (self) -> winner mask
                m1 = sp.tile([128, 1], F32, tag="m1")
                nc.vector.tensor_reduce(m1[:], s96, axis=mybir.AxisListType.X,
                                        op=mybir.AluOpType.max)
                m1x8 = sp.tile([128, 8], F32, tag="m1x8")
                nc.vector.tensor_copy(m1x8[:], neg_inf8[:])
                nc.vector.tensor_copy(m1x8[:, 0:1], m1[:])
                srep = sp.tile([128, NCND], F32, tag="srep")
                nc.vector.match_replace(srep[:], m1x8[:], s96, -1e30)
                w8 = sp.tile([128, 8], F32, tag="w8")
                nc.vector.max(w8[:], srep[:])
                srep2 = sp.tile([128, NCND], F32, tag="srep2")
                nc.vector.match_replace(srep2[:], w8[:], srep[:], 1e30)
                mask96 = mask_all[:, m]
                nc.vector.tensor_scalar(out=mask96, in0=srep2[:],
                                        scalar1=1e29, scalar2=None,
                                        op0=mybir.AluOpType.is_ge)

                # debug: (elem gid + 1) * mask so the test can recover winners
                gid96 = sp.tile([128, T_WIN, BLK], F32, tag="gid96")
                nc.vector.scalar_tensor_tensor(
                    out=gid96[:],
                    in0=wgid_f[:].rearrange("p (t o) -> p t o", o=1
                                            ).to_broadcast((128, T_WIN, BLK)),
                    scalar=float(BLK),
                    in1=iota8f[:].rearrange("p (o b) -> p o b", o=1
                                            ).to_broadcast((128, T_WIN, BLK)),
                    op0=mybir.AluOpType.mult, op1=mybir.AluOpType.add)
                gdbg = sp.tile([128, NCND], F32, tag="gdbg")
                nc.vector.scalar_tensor_tensor(
                    out=gdbg[:],
                    in0=gid96[:].rearrange("p t b -> p (t b)"),
                    scalar=1.0, in1=mask96,
                    op0=mybir.AluOpType.add, op1=mybir.AluOpType.mult)
                gdbg_u = sp.tile([128, NCND], U32, tag="gdbgu")
                nc.vector.tensor_copy(gdbg_u[:], gdbg[:])
                nc.sync.dma_start(out=dbg_gid_d[m], in_=gdbg_u[:])
                nc.sync.dma_start(out=dbg_s_d[m], in_=w8[:])

            pending = None
            for m in range(M_TILES):
                st = phase_a(m)
                if pending is not None:
                    phase_b(pending)
                pending = st
            phase_b(pending)

            # batched epilogue: one sqrt/exp table load for all 8 m-tiles
            s_c = accp.tile([128, M_TILES, NCND], F32, tag="s_c")
            nc.vector.tensor_scalar(out=s_c[:], in0=s_all[:], scalar1=0.0,
                                    scalar2=None, op0=mybir.AluOpType.min)
            dist = accp.tile([128, M_TILES, NCND], F32, tag="dist")
            nc.scalar.activation(dist[:], s_c[:],
                                 mybir.ActivationFunctionType.Sqrt,
                                 bias=0.0, scale=-1.0)
            wexp = accp.tile([128, M_TILES, NCND], F32, tag="wexp")
            nc.scalar.activation(wexp[:], dist[:],
                                 mybir.ActivationFunctionType.Exp,
                                 bias=1.0, scale=-1.0)
            wm = accp.tile([128, M_TILES, NCND], F32, tag="wm")
            nc.vector.tensor_tensor(out=wm[:], in0=wexp[:], in1=mask_all[:],
                                    op=mybir.AluOpType.mult)
            numneg = accp.tile([128, M_TILES], F32, tag="numneg")
            scrap = sp.tile([128, NCND], F32, tag="scrap")
            for m in range(M_TILES):
                nc.vector.scalar_tensor_tensor(
                    out=scrap[:], in0=u_all[:, m], scalar=qf_all[:, m:m + 1],
                    in1=wm[:, m],
                    op0=mybir.AluOpType.subtract, op1=mybir.AluOpType.mult,
                    accum_out=numneg[:, m:m + 1])
            wsum = accp.tile([128, M_TILES], F32, tag="wsum")
            nc.vector.tensor_reduce(wsum[:], wm[:], axis=mybir.AxisListType.X,
                                    op=mybir.AluOpType.add)

            # combine heads: pred = sigmoid(mean_h num_h / den_h)
            nsum2 = sp.tile([128, N_HEADS], F32, tag="nsum2")
            nc.vector.tensor_reduce(
                nsum2[:], numneg[:].rearrange("p (h e) -> p h e", e=2),
                axis=mybir.AxisListType.X, op=mybir.AluOpType.add)
            den = sp.tile([128, N_HEADS], F32, tag="den")
            nc.vector.tensor_reduce(
                den[:], wsum[:].rearrange("p (h e) -> p h e", e=2),
                axis=mybir.AxisListType.X, op=mybir.AluOpType.add)
            den8 = sp.tile([128, N_HEADS], F32, tag="den8")
            nc.vector.tensor_scalar(out=den8[:], in0=den[:],
                                    scalar1=float(N_SENT), scalar2=None,
                                    op0=mybir.AluOpType.add)
            rden = sp.tile([128, N_HEADS], F32, tag="rden")
            nc.vector.reciprocal(rden[:], den8[:])
            ratio = sp.tile([128, N_HEADS], F32, tag="ratio")
            nc.vector.tensor_tensor(out=ratio[:], in0=nsum2[:], in1=rden[:],
                                    op=mybir.AluOpType.mult)
            ssum = sp.tile([128, 1], F32, tag="ssum")
            nc.vector.tensor_reduce(ssum[:], ratio[:], axis=mybir.AxisListType.X,
                                    op=mybir.AluOpType.add)
            preds_s = sp.tile([128, 1], F32, tag="preds")
            nc.scalar.activation(preds_s[:], ssum[:],
                                 mybir.ActivationFunctionType.Sigmoid,
                                 bias=0.0, scale=-1.0 / N_HEADS)
            nc.sync.dma_start(out=preds_d[:], in_=preds_s[:])

    return nc


def _prep_inputs(embeds, field, edges):
    """Host-side layout prep + per-core sharding."""
    embeds = np.asarray(embeds, dtype=np.float32)
    field = np.asarray(field, dtype=np.float32)
    edges = np.asarray(edges)

    en = np.sum(np.square(embeds), axis=-1, dtype=np.float32)
    rhs_aug = np.empty((N_HEADS, KC, N_PAD), dtype=np.float32)
    rhs_aug[:, :DIM, :N_NODES] = embeds.transpose(0, 2, 1)
    rhs_aug[:, DIM, :N_NODES] = en
    rhs_aug[:, DIM + 1, :] = -1.0
    rhs_aug[:, :DIM, N_NODES:] = 0.0
    rhs_aug[:, DIM, N_NODES:] = PAD_EN

    embn = np.zeros((N_HEADS * N_PAD, EW), dtype=np.float32)
    ev = embn.reshape(N_HEADS, N_PAD, EW)
    ev[:, :N_NODES, :DIM] = embeds
    ev[:, :N_NODES, DIM] = en
    ev[:, N_NODES:, DIM] = PAD_EN

    in_maps = []
    for c in range(N_CORES):
        sl = slice(c * EDGES_PER_CORE, (c + 1) * EDGES_PER_CORE)
        qpack = np.zeros((M_TILES, KC, 128), dtype=np.float32)
        aux = np.zeros((M_TILES, 128, 2 + 2 * DIM), dtype=np.float32)
        for m in range(M_TILES):
            h, e = m // 2, m % 2
            nodes = edges[e, sl]
            q = embeds[h, nodes]                      # (128, 32)
            f = field[h, nodes]                       # (128, 32)
            qn = np.einsum('bd,bd->b', q, q)
            qpack[m, :DIM] = (2.0 * q).T
            qpack[m, DIM] = -1.0
            qpack[m, DIM + 1] = qn
            aux[m, :, 0] = qn
            aux[m, :, 1] = np.einsum('bd,bd->b', q, f)
            aux[m, :, 2:2 + DIM] = q
            aux[m, :, 2 + DIM:] = f
        in_maps.append({
            "rhs_aug": rhs_aug, "embn": embn,
            "qpack": qpack, "aux": aux,
        })
    return in_maps


def kernel(embeds, field, edges):
    from concourse.bass_utils import run_bass_kernel_spmd

    nc = _build_program()
    nc.finalize()
    in_maps = _prep_inputs(embeds, field, edges)
    core_ids = list(range(N_CORES))
    trace = bool(os.environ.get("KNN_TRACE"))
    tmpdir = os.environ.get("KNN_TRACE_DIR") or None
    out = run_bass_kernel_spmd(nc, in_maps, core_ids, trace=trace,
                               tmpdir=tmpdir)
    LAST["results"] = out
    preds = np.concatenate(
        [out.results[c]["preds"][:, 0] for c in range(N_CORES)])
    return preds.astype(np.float32)
